# revision 45
# baseline (speedup 1.0000x reference)
"""Trainium2 Bass kernel for nn_CandidateFinder (retrieval_knn).

Reference semantics: for each query row i (batch b), find the ascending list
of key indices j whose binarized 64-bit vector exactly equals the query's
binarized vector; truncate/pad to 64 with -1 (float32 output [B, L, 64]).

Device algorithm (consensus group testing): the host sorts each batch's 4096
keys by their packed 64-bit value and packs each run of K_PACK=4 sorted keys
into ONE test column holding the group's consensus pattern: p_d = +-1 on the
dims where all four keys agree (set E), 0 elsewhere, plus two bias rows
summing to 32-|E|.  With queries encoded +-1 the GEMM score is

    s(i,c) = sum_{d in E_c} q_id * p_cd + (32 - |E_c|) = 32 - 2*disagree,

an exact small integer; s = 32  <=>  q_i agrees with the consensus on all of
E_c, which is implied by q_i exactly matching ANY key of the group (no false
negatives). False positives (q agrees on E_c but is not a group member) are
rare (sorted groups share ~17 consensus dims => ~1e-5/element) and are
resolved exactly on the host with packed-uint64 compares.  This shrinks both
the GEMM and the PSUM-threshold scan 4x vs testing every key individually.

Device work per core (8 cores, data-parallel over the 8192 query rows; the
row batch's 1024 packed columns replicated): fp8e4m3 GEMM [66,1024]x[66,1024]
-> 16 matmuls of 512 cols into PSUM fp32; DVE (is_ge+accum) and ACT
(relu+accum) each drain alternating 2048-col PSUM halves into per-row flag
counts.  Raw Bacc with hand-rolled semaphores; ~10 sem ops per engine keeps
the walrus end-of-NEFF sem-drain ladder short.  Host maps flag hits to the
<=8 candidate rows they cover and recomputes those rows exactly.
"""

import sys
import types

import numpy as np
import ml_dtypes

import concourse.bacc as bacc
import concourse.mybir as mybir
from concourse.bass_utils import run_bass_kernel_spmd

# If BASS_TRACE is set in the environment but the agent image's antenv lacks
# axon_hooks, run_bass_kernel_spmd would crash on import. Provide a None-hook
# shim so tracing degrades to "skipped" instead. (A real hook installed by a
# test harness beforehand is left untouched.)
try:
    from antenv.axon_hooks import get_axon_ntff_profile_hook  # noqa: F401
except ImportError:
    import antenv

    _hooks_mod = types.ModuleType("antenv.axon_hooks")
    _hooks_mod.get_axon_ntff_profile_hook = lambda: None
    _hooks_mod.set_axon_ntff_profile_hook = lambda h: None
    antenv.axon_hooks = _hooks_mod
    sys.modules["antenv.axon_hooks"] = _hooks_mod

B, L, D = 2, 4096, 64
KMAX = 64
N_CORES = 8
ROWS_PER_CORE = (B * L) // N_CORES  # 1024
QBLKS = ROWS_PER_CORE // 128  # 8 query blocks of 128 rows
K_PACK = 4  # keys per consensus test column
NCOL = L // K_PACK  # 1024 packed columns per batch
KDIM = D + 2  # 64 sign rows + 2 bias rows
CHUNK = 512  # matmul moving width (one fp32 PSUM bank)
NFLAG = 9  # reduce-op accumulator columns (per drain unit; unit 7 split)
# flag column -> (kp chunk, covered qblocks)
FLAG_QBS = {c: range(2 * (c % 4), 2 * (c % 4) + 2) for c in range(7)}
FLAG_QBS[7] = range(6, 7)
FLAG_QBS[8] = range(7, 8)
FLAG_CHUNK = {c: c // 4 for c in range(7)}
FLAG_CHUNK[7] = 1
FLAG_CHUNK[8] = 1

_CACHE = {}
LAST_RESULTS = None


# The builder runs from an exec'd string with a fixed pseudo-filename so the
# generated BIR (whose debug frames embed source paths) is byte-identical no
# matter where kernel.py lives -- this keeps the on-disk neuron compile cache
# valid across directories/processes.
_BUILDER_SRC = '''
import concourse.bacc as bacc
import concourse.mybir as mybir

ROWS_PER_CORE = 1024
QBLKS = 8
NCOL = 1024
KDIM = 66
CHUNK = 512
NFLAG = 9
THRESH = 31.0


def _build_nc():
    # The constructor's all_engine_barrier only guards the const-AP memsets
    # (0.0/1.0 etc.), which this kernel never reads -- skip the ~3.5us EVSEM
    # chain it would put at the head of the NEFF.
    import concourse.bass as _bass

    _orig_barrier = _bass.Bass.all_engine_barrier
    _bass.Bass.all_engine_barrier = lambda self, **kw: None
    try:
        nc = bacc.Bacc(
            trn_type="TRN2",
            target_bir_lowering=False,
            disable_frame_to_traceback=True,
        )
    finally:
        _bass.Bass.all_engine_barrier = _orig_barrier

    qst = nc.dram_tensor(
        "qst", [KDIM, ROWS_PER_CORE], mybir.dt.float8e4, kind="ExternalInput"
    )
    kst = nc.dram_tensor(
        "kst", [KDIM, NCOL], mybir.dt.float8e4, kind="ExternalInput"
    )
    flags = nc.dram_tensor(
        "flags", [128, NFLAG], mybir.dt.float32, kind="ExternalOutput"
    )

    from contextlib import ExitStack

    ctx = ExitStack()
    with ctx:
        def sb(name, shape, dt):
            return ctx.enter_context(nc.sbuf_tensor(name, shape, dt))

        def psum(name, shape):
            return ctx.enter_context(
                nc.psum_tensor(name, shape, mybir.dt.float32)
            )

        def sem(name):
            return ctx.enter_context(nc.semaphore(name))

        q_t = sb("q_t", [KDIM, ROWS_PER_CORE], mybir.dt.float8e4)
        kp_t = sb("kp_t", [KDIM, NCOL], mybir.dt.float8e4)
        warm = sb("warmt", [KDIM, 128], mybir.dt.float8e4)
        tr_d = sb("tr_d", [128, 2048], mybir.dt.bfloat16)
        tr_a = sb("tr_a", [128, 2048], mybir.dt.bfloat16)
        fl = sb("fl", [128, NFLAG], mybir.dt.float32)
        act_bias = sb("act_bias", [128, 1], mybir.dt.float32)
        ps0 = psum("ps0", [128, 2048])
        ps1 = psum("ps1", [128, 2048])

        dma_q = sem("dma_q")  # all q partitions -> 32
        dma_k = sem("dma_k")  # all kp partitions -> 32
        mm = sem("mm")  # PE: drain unit u of the score stream done -> >= u+1
        red_d = sem("red_d")  # DVE reduce ops done -> count
        red_a = sem("red_a")  # ACT reduce ops done -> count
        setup = sem("setup")  # DVE memset of act_bias done
        dma_out = sem("dma_out")  # flags store issued (drained by epilogue)

        # --- straight-line, single-basic-block program.

        # the ACT bias constant (DVE is idle until the first PSUM half lands)
        nc.vector.memset(act_bias[:], -THRESH).then_inc(setup, 1)

        # inputs balanced over the three HWDGE queues (~44 1KB descriptors
        # each); the descriptor rate per queue, not bytes, is the bottleneck.
        nc.sync.dma_start(out=kp_t[0:44, :], in_=kst[0:44, :]).then_inc(
            dma_k, 16
        )
        nc.gpsimd.dma_start(out=q_t[0:44, :], in_=qst[0:44, :]).then_inc(
            dma_q, 16
        )
        nc.scalar.dma_start(out=kp_t[44:KDIM, :], in_=kst[44:KDIM, :]).then_inc(
            dma_k, 16
        )
        nc.scalar.dma_start(out=q_t[44:KDIM, :], in_=qst[44:KDIM, :]).then_inc(
            dma_q, 16
        )

        # flags store on the sync queue once both reducers are done. The
        # walrus epilogue's per-engine DRAIN flushes the queue before the
        # NEFF retires, so no completion wait is needed.
        nc.sync.wait_ge(red_a, 4)
        nc.sync.wait_ge(red_d, 5)
        nc.sync.dma_start(out=flags[:], in_=fl[:]).then_inc(dma_out, 16)
        _ = dma_out

        # tensor: warm the PE pstate clock during the DMA head with dummy
        # matmuls on an un-DMA'd scratch tile; they land in ps1's last bank,
        # which the first real matmul there (start=True) resets.
        for _ in range(4):
            nc.tensor.matmul(
                ps1[:, 1536:1664],
                warm[:, 0:128],
                warm[:, 0:128],
                start=True,
                stop=True,
                skip_group_check=True,
            )

        # tensor: the real score stream. 16 matmuls, 8 drain units of 1024
        # cols (unit u = fill u//4, qb pair u%4): u even -> lower bank pair
        # (DVE), u odd -> upper bank pair (ACT). mm counts finished units;
        # fill-1 units wait for their unit's fill-0 drain.
        nc.tensor.wait_ge(dma_q, 32)
        nc.tensor.wait_ge(dma_k, 32)
        for c in range(2):
            for qb in range(QBLKS):
                m = c * QBLKS + qb
                if m == 8:
                    nc.tensor.wait_ge(red_d, 1)  # u0's banks free
                if m == 10:
                    nc.tensor.wait_ge(red_a, 1)  # u1's banks free
                if m == 12:
                    nc.tensor.wait_ge(red_d, 2)  # u2's banks free
                if m == 14:
                    nc.tensor.wait_ge(red_a, 2)  # u3's banks free
                ps = ps0 if qb < 4 else ps1
                j0 = (qb % 4) * CHUNK
                mmi = nc.tensor.matmul(
                    ps[:, j0 : j0 + CHUNK],
                    q_t[:, qb * 128 : (qb + 1) * 128],
                    kp_t[:, c * CHUNK : (c + 1) * CHUNK],
                    start=True,
                    stop=True,
                )
                if m % 2 == 1:
                    mmi.then_inc(mm, 1)

        # scalar (ACT): preload the activation table during the DMA head,
        # then drain the odd units (upper bank pairs).
        nc.scalar.wait_ge(setup, 1)
        nc.scalar.activation(
            out=tr_a[:, 0:1],
            in_=act_bias[:],
            func=mybir.ActivationFunctionType.Relu,
            bias=act_bias[:],
            scale=1.0,
        )
        for i in range(3):
            u = 2 * i + 1  # units 1, 3, 5
            ps = ps0 if (u % 4) < 2 else ps1
            nc.scalar.wait_ge(mm, u + 1)
            nc.scalar.activation(
                out=tr_a[:, 0:1024],
                in_=ps[:, 1024:2048],
                func=mybir.ActivationFunctionType.Relu,
                bias=act_bias[:],
                scale=1.0,
                accum_out=fl[:, u : u + 1],
            ).then_inc(red_a, 1)
        # unit 7 (last, on the critical tail) splits across both reducers:
        # ACT takes its upper bank (qb 7), DVE the lower (qb 6).
        nc.scalar.wait_ge(mm, 8)
        nc.scalar.activation(
            out=tr_a[:, 0:CHUNK],
            in_=ps1[:, 1536:2048],
            func=mybir.ActivationFunctionType.Relu,
            bias=act_bias[:],
            scale=1.0,
            accum_out=fl[:, 8:9],
        ).then_inc(red_a, 1)

        # vector (DVE): drains the even units (lower bank pairs) and the
        # lower bank of unit 7.
        def dve_red(src, col, n):
            nc.vector.tensor_scalar(
                out=tr_d[:, 0:n],
                in0=src,
                scalar1=THRESH,
                scalar2=0.0,
                op0=mybir.AluOpType.is_ge,
                op1=mybir.AluOpType.add,
                accum_out=fl[:, col : col + 1],
            ).then_inc(red_d, 1)

        for i in range(4):
            u = 2 * i  # units 0, 2, 4, 6
            ps = ps0 if (u % 4) < 2 else ps1
            nc.vector.wait_ge(mm, u + 1)
            dve_red(ps[:, 0:1024], u, 1024)
        nc.vector.wait_ge(mm, 8)
        dve_red(ps1[:, 1024:1536], 7, CHUNK)

    nc.finalize()
    return nc
'''

_builder_mod = types.ModuleType("cf_builder")
exec(compile(_BUILDER_SRC, "<cf_builder>", "exec"), _builder_mod.__dict__)
_build_nc = _builder_mod._build_nc


def _get_nc():
    if "nc" not in _CACHE:
        _CACHE["nc"] = _build_nc()
    return _CACHE["nc"]


def _pack_u64(bits):
    """[N, 64] bool -> [N] uint64 (bit d = bits[:, d])."""
    return (
        np.packbits(bits, axis=1, bitorder="little")
        .view("<u8")
        .reshape(-1)
    )


def _group_keys(k_bits):
    """Sort keys, pack runs of K_PACK=4 into consensus columns.

    Returns kp_enc [KDIM, NCOL] float32 (to be cast fp8) and, for the host
    flag model, the groups' consensus masks/patterns as uint64.
    """
    u = _pack_u64(k_bits)
    order = np.argsort(u, kind="stable")
    gb = k_bits[order].reshape(NCOL, K_PACK, D)
    all1 = gb.all(axis=1)  # [NCOL, D]
    all0 = (~gb).all(axis=1)
    pat = all1.astype(np.float32) - all0.astype(np.float32)
    e_cnt = (all1 | all0).sum(axis=1).astype(np.int32)
    bias = (D // 2) - e_cnt  # 32 - |E|, in [-32, 32]
    b1 = np.clip(bias, -16, 16)
    b2 = bias - b1
    kp_enc = np.concatenate(
        [
            pat.T,
            b1[None, :].astype(np.float32),
            b2[None, :].astype(np.float32),
        ],
        axis=0,
    )  # [66, NCOL]
    mask_u64 = _pack_u64(all1 | all0)
    pat_u64 = _pack_u64(all1)
    return kp_enc, mask_u64, pat_u64


def _exact_rows(q_bits_rows, k_u64, out_rows):
    """Exact reference semantics for a set of query rows (uint64 compare).

    out_rows comes in pre-filled with -1; only rows with a real match (none,
    for random inputs) take the slow path.
    """
    qu = _pack_u64(q_bits_rows)  # [R]
    eq = qu[:, None] == k_u64[None, :]  # [R, 4096]
    for r in np.nonzero(eq.any(axis=1))[0]:
        idx = np.nonzero(eq[r])[0][:KMAX]
        out_rows[r, : idx.size] = idx.astype(np.float32)
    return out_rows


def kernel(query_up, key_up, head_idx=0):
    global LAST_RESULTS
    q = np.asarray(query_up, dtype=np.float32)  # [B, L, D]
    k = np.asarray(key_up, dtype=np.float32)
    assert q.shape == (B, L, D) and k.shape == (B, L, D)

    q_bits = q > 0
    k_bits = k > 0

    # Host prep: queries as +-1 (+ two ones rows for the bias dims),
    # transposed to [KDIM, rows]; keys packed into consensus columns.
    f8 = ml_dtypes.float8_e4m3fn
    qs = np.where(q_bits, np.float32(1.0), np.float32(-1.0))
    ones = np.ones((B, L, 2), dtype=np.float32)
    q_enc = np.concatenate([qs, ones], axis=2)  # [B, L, 66]

    kp_enc = []
    for b in range(B):
        enc, _, _ = _group_keys(k_bits[b])
        kp_enc.append(np.ascontiguousarray(enc).astype(f8))

    in_maps = []
    for c in range(N_CORES):
        b = c // (N_CORES // B)
        s = (c % (N_CORES // B)) * ROWS_PER_CORE
        qct = np.ascontiguousarray(
            q_enc[b, s : s + ROWS_PER_CORE].T
        ).astype(f8)
        in_maps.append({"qst": qct, "kst": kp_enc[b]})

    nc = _get_nc()
    res = run_bass_kernel_spmd(nc, in_maps, core_ids=list(range(N_CORES)))
    LAST_RESULTS = res

    out = np.full((B, L, KMAX), -1.0, dtype=np.float32)
    k_u64 = [_pack_u64(k_bits[b]) for b in range(B)]
    for c in range(N_CORES):
        b = c // (N_CORES // B)
        s = (c % (N_CORES // B)) * ROWS_PER_CORE
        fl = res.results[c]["flags"]  # [128, NFLAG]
        ps_, hs = np.nonzero(fl[:, :NFLAG] > 0.1)
        if ps_.size:
            rows = set()
            for p, h in zip(ps_, hs):
                for qb in FLAG_QBS[int(h)]:
                    rows.add(s + qb * 128 + int(p))
            rows = sorted(rows)
            patched = np.full((len(rows), KMAX), -1.0, dtype=np.float32)
            _exact_rows(q_bits[b][rows], k_u64[b], patched)
            out[b, rows] = patched

    return out
